# revision 15
# baseline (speedup 1.0000x reference)
"""ContextRetentionLayer Trainium2 kernel.

Reference computation (per token t, d=1024, W=512 memory slots):
    s[t, w]   = (x[t] . mb[w]) / 32
    attn[t]   = softmax_w(s[t])
    r[t]      = sum_w attn[t, w] * mb[w]
    g[t]      = sigmoid(x[t] @ gw.T + gb)
    out[t]    = g[t] * x[t] + (1 - g[t]) * r[t]

Sharding: 4x4096 = 16384 tokens split evenly across 8 cores (2048 each);
memory_bank / gate weights replicated.

Device-side layout is fully transposed (d on partitions, tokens on the free
axis) so every matmul contracts along the partition dim with no on-chip
transposes:
    sT[w, t]  = sum_d mbT[d, w] * xT[d, t]      (lhsT = mbT chunk)
    den[t]    = sum_w exp(sT)[w, t]             (lhsT = ones column)
    rT[d, t]  = sum_w mb[w, d] * attnT[w, t]    (lhsT = mb chunk, natural)
    zT[e, t]  = sum_d gwT[d, e] * xT[d, t]      (lhsT = gwT chunk)
softmax reduces over w via the ones-matmul; the reciprocal runs on VectorE
(reciprocal_approx_accurate) and is broadcast across partitions on GpSimd.
mbT is derived on-chip by PE-transposing mb (saves a 2MB load on the critical
startup path). Skipping the softmax max-subtraction is safe: scores/32 is
~N(0,1) here, far from fp32 overflow.

Matmul tensors are float32r end-to-end (full PE rate at moving dim >= 256;
plain fp32 matmul is 4 cycles/row). Measured on HW: ~114 us/core steady-state,
~= the 109 us pure-matmul floor; rel err vs fp32 reference ~2e-4.
"""

import numpy as np

import concourse.bass as bass
import concourse.tile as tile
from concourse import bacc, bass_utils, mybir
from concourse.bass import ts

AF = mybir.ActivationFunctionType
F32 = mybir.dt.float32
F32R = mybir.dt.float32r

N_CORES = 8
B, S, D = 4, 4096, 1024
W = 512
T_CORE = (B * S) // N_CORES  # 2048 tokens per core
T_TILE = 512                 # moving free dim per matmul (fp32 max)
NT = T_CORE // T_TILE        # 4 token tiles
DC = D // 128                # 8 chunks of the embed dim
WC = W // 128                # 4 chunks of the memory window


def _body(tc: tile.TileContext, reps: int = 1):
    nc = tc.nc

    xT = nc.dram_tensor("xt", (D, T_CORE), F32R, kind="ExternalInput").ap()
    mb = nc.dram_tensor("mb", (W, D), F32R, kind="ExternalInput").ap()
    gwT = nc.dram_tensor("gwt", (D, D), F32R, kind="ExternalInput").ap()
    gb = nc.dram_tensor("gb", (D,), F32, kind="ExternalInput").ap()
    ones_d = nc.dram_tensor("ones", (128, 1), F32R, kind="ExternalInput").ap()
    ident_d = nc.dram_tensor("ident", (128, 128), F32R, kind="ExternalInput").ap()
    outT = nc.dram_tensor("outt", (D, T_CORE), F32, kind="ExternalOutput").ap()

    for _rep in range(reps):
        _emit_once(tc, xT, mb, gwT, gb, ones_d, ident_d, outT)


def _emit_once(tc, xT, mb, gwT, gb, ones_d, ident_d, outT):
    nc = tc.nc
    with (
        tc.tile_pool(name="const", bufs=1) as const,
        tc.tile_pool(name="big", bufs=1) as big,
        tc.tile_pool(name="work", bufs=3) as work,
        tc.tile_pool(name="mm_ps", bufs=6, space="PSUM") as mm_ps,
        tc.tile_pool(name="den_psp", bufs=2, space="PSUM") as den_psp,
    ):
        # ---- tiles: constants (replicated weights) + resident transposed x.
        # All large loads are chunked and emitted in need-order so the PE's
        # first dependencies (mbT + x tile 0) land first; the pass-2 weights
        # (mb, gwT) stream in behind the remaining x tiles.
        mbT_s = const.tile([128, DC, W], F32R)
        mb_s = const.tile([128, WC, D], F32R)
        gwT_s = const.tile([128, DC, D], F32R)
        gb_s = const.tile([128, DC], F32)
        ones_s = const.tile([128, 1], F32R)
        x_s = big.tile([128, DC, T_CORE], F32R)

        mbv = mb.rearrange("(c p) d -> p c d", p=128)
        gwTv = gwT.rearrange("(c p) e -> p c e", p=128)
        xTv = xT.rearrange("(c p) t -> p c t", p=128)

        def load_x(ti):
            nc.sync.dma_start(
                out=x_s[:, :, ts(ti, T_TILE)], in_=xTv[:, :, ts(ti, T_TILE)]
            )

        # need-ordered loads: tiny constants first, then mb (feeds the
        # on-chip transpose for pass 1 AND pass 2's retrieved matmul) and
        # x tile 0; gwT (pass 2 gate) last.
        ident = const.tile([128, 128], F32R)
        nc.sync.dma_start(out=ident, in_=ident_d)
        for wc in range(WC):
            nc.sync.dma_start(out=mb_s[:, wc, :], in_=mbv[:, wc, :])
        load_x(0)
        nc.sync.dma_start(out=ones_s, in_=ones_d)
        nc.sync.dma_start(out=gb_s, in_=gb.rearrange("(c p) -> p c", p=128))

        # mbT = mb.T via PE transpose (f32r, 1.5 cyc/row); DVE copies the
        # PSUM result out, rounding to f32r.
        for wc in range(WC):
            for dc in range(DC):
                t_ps = mm_ps.tile([128, 128], F32R, tag="mm")
                nc.tensor.transpose(t_ps, mb_s[:, wc, ts(dc, 128)], ident)
                nc.vector.tensor_copy(mbT_s[:, dc, ts(wc, 128)], t_ps)

        load_x(1)
        load_x(2)
        load_x(3)
        for dc in range(DC):
            nc.sync.dma_start(out=gwT_s[:, dc, :], in_=gwTv[:, dc, :])

        at_s = big.tile([128, WC, T_CORE], F32R)  # exp(s/32), then attn in place
        rd_s = big.tile([1, T_CORE], F32)         # 1 / denominator
        rb_s = big.tile([128, T_CORE], F32)       # broadcast across partitions

        # ---- pass 1: scores, exp, denominators, attn normalize
        for ti in range(NT):
            tsl = ts(ti, T_TILE)
            den_ps = den_psp.tile([1, T_TILE], F32, tag="den")
            for wc in range(WC):
                s_ps = mm_ps.tile([128, T_TILE], F32, tag="mm")
                for dc in range(DC):
                    nc.tensor.matmul(
                        s_ps,
                        lhsT=mbT_s[:, dc, ts(wc, 128)],
                        rhs=x_s[:, dc, tsl],
                        start=(dc == 0),
                        stop=(dc == DC - 1),
                    )
                nc.scalar.activation(
                    out=at_s[:, wc, tsl], in_=s_ps, func=AF.Exp, scale=1.0 / 32.0
                )
                nc.tensor.matmul(
                    den_ps,
                    lhsT=ones_s,
                    rhs=at_s[:, wc, tsl],
                    start=(wc == 0),
                    stop=(wc == WC - 1),
                )
            rscr = work.tile([1, T_TILE], F32, tag="rscr")
            nc.vector.reciprocal_approx_accurate(
                out=rd_s[:, tsl], in_=den_ps, scratch=rscr
            )
            nc.gpsimd.partition_broadcast(rb_s[:, tsl], rd_s[:, tsl])
            for wc in range(WC):
                nc.vector.tensor_mul(at_s[:, wc, tsl], at_s[:, wc, tsl], rb_s[:, tsl])

        # ---- pass 2: retrieved, gate, combine
        outv = outT.rearrange("(c p) t -> p c t", p=128)
        for ti in range(NT):
            tsl = ts(ti, T_TILE)
            for dc in range(DC):
                z_ps = mm_ps.tile([128, T_TILE], F32, tag="mm")
                for kc in range(DC):
                    nc.tensor.matmul(
                        z_ps,
                        lhsT=gwT_s[:, kc, ts(dc, 128)],
                        rhs=x_s[:, kc, tsl],
                        start=(kc == 0),
                        stop=(kc == DC - 1),
                    )
                g = work.tile([128, T_TILE], F32, tag="g")
                nc.scalar.activation(
                    out=g, in_=z_ps, func=AF.Sigmoid, bias=gb_s[:, dc : dc + 1]
                )
                r_ps = mm_ps.tile([128, T_TILE], F32, tag="mm")
                for wc in range(WC):
                    nc.tensor.matmul(
                        r_ps,
                        lhsT=mb_s[:, wc, ts(dc, 128)],
                        rhs=at_s[:, wc, tsl],
                        start=(wc == 0),
                        stop=(wc == WC - 1),
                    )
                o = work.tile([128, T_TILE], F32, tag="o")
                nc.vector.tensor_sub(o, x_s[:, dc, tsl].bitcast(F32), r_ps)
                nc.vector.tensor_mul(o, o, g)
                nc.vector.tensor_add(o, o, r_ps)
                nc.sync.dma_start(out=outv[:, dc, tsl], in_=o)


_NC_CACHE = None


def _build_nc(reps: int = 1):
    global _NC_CACHE
    if reps == 1 and _NC_CACHE is not None:
        return _NC_CACHE
    nc = bacc.Bacc("TRN2", target_bir_lowering=False, debug=False,
                   enable_asserts=False)
    with tile.TileContext(nc) as tc:
        _body(tc, reps)
    nc.compile()
    if reps == 1:
        _NC_CACHE = nc
    return nc


def make_in_maps(x, memory_bank, gate_w, gate_b):
    x = np.ascontiguousarray(np.asarray(x, np.float32)).reshape(B * S, D)
    mb_n = np.ascontiguousarray(np.asarray(memory_bank, np.float32))
    gwT_n = np.ascontiguousarray(np.asarray(gate_w, np.float32).T)
    gb_n = np.ascontiguousarray(np.asarray(gate_b, np.float32))
    in_maps = []
    for c in range(N_CORES):
        xs = x[c * T_CORE : (c + 1) * T_CORE]
        in_maps.append(
            {
                "xt": np.ascontiguousarray(xs.T),
                "mb": mb_n,
                "gwt": gwT_n,
                "gb": gb_n,
                "ones": np.ones((128, 1), np.float32),
                "ident": np.eye(128, dtype=np.float32),
            }
        )
    return in_maps


def assemble_out(results):
    shards = [results[c]["outt"].T for c in range(N_CORES)]
    return np.concatenate(shards, axis=0).reshape(B, S, D).astype(np.float32)


def kernel(x, memory_bank, gate_w, gate_b, _run_kwargs=None):
    nc = _build_nc()
    in_maps = make_in_maps(x, memory_bank, gate_w, gate_b)
    res = bass_utils.run_bass_kernel_spmd(
        nc, in_maps, core_ids=list(range(N_CORES)), **(_run_kwargs or {})
    )
    out = assemble_out(res.results)
    if _run_kwargs:
        kernel.last_result = res
    return out


# revision 17
# speedup vs baseline: 1.1526x; 1.1526x over previous
"""ContextRetentionLayer Trainium2 kernel.

Reference computation (per token t, d=1024, W=512 memory slots):
    s[t, w]   = (x[t] . mb[w]) / 32
    attn[t]   = softmax_w(s[t])
    r[t]      = sum_w attn[t, w] * mb[w]
    g[t]      = sigmoid(x[t] @ gw.T + gb)
    out[t]    = g[t] * x[t] + (1 - g[t]) * r[t]

Sharding: 4x4096 = 16384 tokens split evenly across 8 cores (2048 each);
memory_bank / gate weights replicated.

Device-side layout is fully transposed (d on partitions, tokens on the free
axis) so every matmul contracts along the partition dim with no on-chip
transposes:
    sT[w, t]  = sum_d mbT[d, w] * xT[d, t]      (lhsT = mbT chunk)
    den[t]    = sum_w exp(sT)[w, t]             (lhsT = ones column)
    rT[d, t]  = sum_w mb[w, d] * attnT[w, t]    (lhsT = mb chunk, natural)
    zT[e, t]  = sum_d gwT[d, e] * xT[d, t]      (lhsT = gwT chunk)
softmax reduces over w via the ones-matmul; the reciprocal runs on VectorE
(reciprocal_approx_accurate) and is broadcast across partitions on GpSimd.
mbT is derived on-chip by PE-transposing mb (saves a 2MB load on the critical
startup path). Skipping the softmax max-subtraction is safe: scores/32 is
~N(0,1) here, far from fp32 overflow.

Matmul tensors are float32r end-to-end (full PE rate at moving dim >= 256;
plain fp32 matmul is 4 cycles/row). Measured on HW: ~114 us/core steady-state,
~= the 109 us pure-matmul floor; rel err vs fp32 reference ~2e-4.
"""

import numpy as np

import concourse.bass as bass
import concourse.tile as tile
from concourse import bacc, bass_utils, mybir
from concourse.bass import ts

AF = mybir.ActivationFunctionType
F32 = mybir.dt.float32
F32R = mybir.dt.float32r

N_CORES = 8
B, S, D = 4, 4096, 1024
W = 512
T_CORE = (B * S) // N_CORES  # 2048 tokens per core
T_TILE = 512                 # moving free dim per matmul (fp32 max)
NT = T_CORE // T_TILE        # 4 token tiles
DC = D // 128                # 8 chunks of the embed dim
WC = W // 128                # 4 chunks of the memory window


def _body(tc: tile.TileContext, reps: int = 1):
    nc = tc.nc

    xT = nc.dram_tensor("xt", (D, T_CORE), F32R, kind="ExternalInput").ap()
    mb = nc.dram_tensor("mb", (W, D), F32R, kind="ExternalInput").ap()
    gwT = nc.dram_tensor("gwt", (D, D), F32R, kind="ExternalInput").ap()
    gb = nc.dram_tensor("gb", (D,), F32, kind="ExternalInput").ap()
    ones_d = nc.dram_tensor("ones", (128, 1), F32R, kind="ExternalInput").ap()
    ident_d = nc.dram_tensor("ident", (128, 128), F32R, kind="ExternalInput").ap()
    outT = nc.dram_tensor("outt", (D, T_CORE), F32, kind="ExternalOutput").ap()

    for _rep in range(reps):
        _emit_once(tc, xT, mb, gwT, gb, ones_d, ident_d, outT)


def _emit_once(tc, xT, mb, gwT, gb, ones_d, ident_d, outT):
    nc = tc.nc
    with (
        tc.tile_pool(name="const", bufs=1) as const,
        tc.tile_pool(name="big", bufs=1) as big,
        tc.tile_pool(name="work", bufs=3) as work,
        tc.tile_pool(name="mm_ps", bufs=6, space="PSUM") as mm_ps,
        tc.tile_pool(name="den_psp", bufs=2, space="PSUM") as den_psp,
    ):
        # ---- tiles: constants (replicated weights) + resident transposed x.
        # All large loads are chunked and emitted in need-order so the PE's
        # first dependencies (mbT + x tile 0) land first; the pass-2 weights
        # (mb, gwT) stream in behind the remaining x tiles.
        mbT_s = const.tile([128, DC, W], F32R)
        mb_s = const.tile([128, WC, D], F32R)
        gwT_s = const.tile([128, DC, D], F32R)
        gb_s = const.tile([128, DC], F32)
        ones_s = const.tile([128, 1], F32R)
        x_s = big.tile([128, DC, T_CORE], F32R)

        mbv = mb.rearrange("(c p) d -> p c d", p=128)
        gwTv = gwT.rearrange("(c p) e -> p c e", p=128)
        xTv = xT.rearrange("(c p) t -> p c t", p=128)

        def load_x(ti):
            nc.sync.dma_start(
                out=x_s[:, :, ts(ti, T_TILE)], in_=xTv[:, :, ts(ti, T_TILE)]
            )

        # need-ordered loads: ident (gates the transposes), then mb (feeds
        # the on-chip transpose for pass 1 AND pass 2's retrieved matmul) and
        # x tile 0; gwT (pass 2 gate) last.
        ident = const.tile([128, 128], F32R)
        nc.sync.dma_start(out=ident, in_=ident_d)
        for wc in range(WC):
            nc.sync.dma_start(out=mb_s[:, wc, :], in_=mbv[:, wc, :])
        load_x(0)
        nc.sync.dma_start(out=ones_s, in_=ones_d)
        nc.sync.dma_start(out=gb_s, in_=gb.rearrange("(c p) -> p c", p=128))

        # mbT = mb.T via PE transpose (f32r, 1.5 cyc/row); DVE copies the
        # PSUM result out, rounding to f32r.
        for wc in range(WC):
            for dc in range(DC):
                t_ps = mm_ps.tile([128, 128], F32R, tag="mm")
                nc.tensor.transpose(t_ps, mb_s[:, wc, ts(dc, 128)], ident)
                nc.vector.tensor_copy(mbT_s[:, dc, ts(wc, 128)], t_ps)

        load_x(1)
        load_x(2)
        load_x(3)
        for dc in range(DC):
            nc.sync.dma_start(out=gwT_s[:, dc, :], in_=gwTv[:, dc, :])

        at_s = big.tile([128, WC, T_CORE], F32R)  # exp(s/32), then attn in place
        rd_s = big.tile([1, T_CORE], F32)         # 1 / denominator
        rb_s = big.tile([128, T_CORE], F32)       # broadcast across partitions

        # ---- pass 1: scores, exp, denominators, attn normalize
        for ti in range(NT):
            tsl = ts(ti, T_TILE)
            den_ps = den_psp.tile([1, T_TILE], F32, tag="den")
            for wc in range(WC):
                s_ps = mm_ps.tile([128, T_TILE], F32, tag="mm")
                for dc in range(DC):
                    nc.tensor.matmul(
                        s_ps,
                        lhsT=mbT_s[:, dc, ts(wc, 128)],
                        rhs=x_s[:, dc, tsl],
                        start=(dc == 0),
                        stop=(dc == DC - 1),
                    )
                nc.scalar.activation(
                    out=at_s[:, wc, tsl], in_=s_ps, func=AF.Exp, scale=1.0 / 32.0
                )
                nc.tensor.matmul(
                    den_ps,
                    lhsT=ones_s,
                    rhs=at_s[:, wc, tsl],
                    start=(wc == 0),
                    stop=(wc == WC - 1),
                )
            rscr = work.tile([1, T_TILE], F32, tag="rscr")
            nc.vector.reciprocal_approx_accurate(
                out=rd_s[:, tsl], in_=den_ps, scratch=rscr
            )
            nc.gpsimd.partition_broadcast(rb_s[:, tsl], rd_s[:, tsl])
            for wc in range(WC):
                nc.vector.tensor_mul(at_s[:, wc, tsl], at_s[:, wc, tsl], rb_s[:, tsl])

        # ---- pass 2: retrieved, gate, combine. The final (ti, dc) iteration
        # is split into half-width slices so the post-PE combine/store tail is
        # shorter before the kernel drain.
        outv = outT.rearrange("(c p) t -> p c t", p=128)

        def p2_iter(dc, t0, tw):
            tsl = slice(t0, t0 + tw)
            z_ps = mm_ps.tile([128, tw], F32, tag="mm")
            for kc in range(DC):
                nc.tensor.matmul(
                    z_ps,
                    lhsT=gwT_s[:, kc, ts(dc, 128)],
                    rhs=x_s[:, kc, tsl],
                    start=(kc == 0),
                    stop=(kc == DC - 1),
                )
            g = work.tile([128, tw], F32, tag="g")
            nc.scalar.activation(
                out=g, in_=z_ps, func=AF.Sigmoid, bias=gb_s[:, dc : dc + 1]
            )
            r_ps = mm_ps.tile([128, tw], F32, tag="mm")
            for wc in range(WC):
                nc.tensor.matmul(
                    r_ps,
                    lhsT=mb_s[:, wc, ts(dc, 128)],
                    rhs=at_s[:, wc, tsl],
                    start=(wc == 0),
                    stop=(wc == WC - 1),
                )
            o = work.tile([128, tw], F32, tag="o")
            nc.vector.tensor_sub(o, x_s[:, dc, tsl].bitcast(F32), r_ps)
            nc.vector.tensor_mul(o, o, g)
            nc.vector.tensor_add(o, o, r_ps)
            nc.sync.dma_start(out=outv[:, dc, tsl], in_=o)

        for ti in range(NT):
            for dc in range(DC):
                if ti == NT - 1 and dc == DC - 1:
                    p2_iter(dc, ti * T_TILE, T_TILE // 2)
                    p2_iter(dc, ti * T_TILE + T_TILE // 2, T_TILE // 2)
                else:
                    p2_iter(dc, ti * T_TILE, T_TILE)


_NC_CACHE = None


def _build_nc(reps: int = 1):
    global _NC_CACHE
    if reps == 1 and _NC_CACHE is not None:
        return _NC_CACHE
    nc = bacc.Bacc("TRN2", target_bir_lowering=False, debug=False,
                   enable_asserts=False)
    with tile.TileContext(nc) as tc:
        _body(tc, reps)
    nc.compile()
    if reps == 1:
        _NC_CACHE = nc
    return nc


def make_in_maps(x, memory_bank, gate_w, gate_b):
    x = np.ascontiguousarray(np.asarray(x, np.float32)).reshape(B * S, D)
    mb_n = np.ascontiguousarray(np.asarray(memory_bank, np.float32))
    gwT_n = np.ascontiguousarray(np.asarray(gate_w, np.float32).T)
    gb_n = np.ascontiguousarray(np.asarray(gate_b, np.float32))
    in_maps = []
    for c in range(N_CORES):
        xs = x[c * T_CORE : (c + 1) * T_CORE]
        in_maps.append(
            {
                "xt": np.ascontiguousarray(xs.T),
                "mb": mb_n,
                "gwt": gwT_n,
                "gb": gb_n,
                "ones": np.ones((128, 1), np.float32),
                "ident": np.eye(128, dtype=np.float32),
            }
        )
    return in_maps


def assemble_out(results):
    shards = [results[c]["outt"].T for c in range(N_CORES)]
    return np.concatenate(shards, axis=0).reshape(B, S, D).astype(np.float32)


def kernel(x, memory_bank, gate_w, gate_b, _run_kwargs=None):
    nc = _build_nc()
    in_maps = make_in_maps(x, memory_bank, gate_w, gate_b)
    res = bass_utils.run_bass_kernel_spmd(
        nc, in_maps, core_ids=list(range(N_CORES)), **(_run_kwargs or {})
    )
    out = assemble_out(res.results)
    if _run_kwargs:
        kernel.last_result = res
    return out
